# revision 7
# baseline (speedup 1.0000x reference)
"""BitLinear (activation int8-quant + ternary weight) + squared-ReLU on 8 Trainium2
NeuronCores.

Sharding: tensor-parallel over weight rows (out_features). Each core receives the
full activation tensor and a 1/8 slice of the weight matrix, computes its slice of
the GEMM + squared ReLU, and the host concatenates the slices.

v2 layout of work (vs the first working version):
  - All transposes (x_q tiles and w_q row-tiles) run on the DMA xbar
    (dma_start_transpose, 2-byte dtype) instead of the PE, so the PE runs the
    bf16 GEMM stream only.
  - w is DMAed once and kept resident in SBUF as f32; the quantization pass
    reads it from SBUF after the w_scale AllReduce (no second HBM pass).
  - The w_scale chain (|w| partial sums -> partition sum via a tiny fp32
    matmul -> scalar AllReduce -> thresholds -> chunk-0 quantize+transpose) is
    emitted under tc.high_priority() so the scheduler runs it as early as the
    data allows.
  - Two HWDGE rings split the DMA traffic: sync carries x in + out writes,
    scalar carries w in + all xbar transposes.
  - Engine balance: gpsimd does the per-token amax for steady-state tiles
    (DVE covers the head tiles while gpsimd is blocked on the collective),
    ACT does the x*(127/amax) scale, DVE does the exact rounding, the scale
    scalars, the w compares, and the output drain.
  - Output drain is relu(g*psum)^2 computed as a = max(psum*g, 0) (one
    tensor_scalar) then a*a (one tensor_tensor), written per 512-wide chunk.
  - The first HEAD_C0 tiles' chunk-0 GEMMs are emitted before any chunk-1
    work so the PE can start as soon as the first half of the weights is
    quantized.

Math notes (unchanged):
  - x_q = round(x * 127/scale), scale = clip(amax_row(|x|), 1e-5). Values are
    integers in [-127, 127] -> exact in bf16.
  - w_q in {-1, 0, 1} = (w > 0.5*ws) - (w < -0.5*ws) via exact fp32 strict
    compares; ws = mean(|W|) over the full weight (AllReduce of per-core sums).
  - bf16 GEMM with fp32 PSUM accumulation is exact (integer products, partial
    sums < 2^24).
  - Rounding uses the +1.5*2^23 magic-constant trick after the product is
    rounded to fp32 (same double-rounding as the reference).
"""

import sys

if "/opt/trn_rl_repo" not in sys.path:
    sys.path.insert(0, "/opt/trn_rl_repo")

import numpy as np

import concourse.bacc as bacc
import concourse.mybir as mybir
import concourse.tile as tile
from concourse.bass_utils import run_bass_kernel_spmd
from concourse.masks import make_identity
from concourse.tile import add_dep_helper

dt = mybir.dt
Alu = mybir.AluOpType
NCORES = 8
C_MAGIC = 1.5 * 2**23  # fp32 round-to-nearest-even forcing constant
HEAD = 8               # x tiles pre-processed during the weight phase
HEAD_C0 = 6            # head tiles whose chunk-0 GEMM runs before chunk 1 exists
WARMUP_MM = 40         # HAM warmup matmuls between the collective and the GEMM

# Stash of the most recent BassKernelResults (test harness reads exec_time_ns).
LAST_RESULTS = None

_NC_CACHE = {}


def _build(T, K, O, max_val):
    """Build + compile the per-core Bass module.

    Per-core tensors: x [T, K] f32 (replicated), w [O, K] f32 (this core's rows),
    out [T, O] f32.
    """
    assert T % 128 == 0 and K % 128 == 0 and O % 512 == 0
    TT = T // 128     # token tiles
    KT = K // 128     # contraction tiles
    OC = O // 512     # psum-width output chunks per core
    OT = O // 128     # weight row tiles
    n_w_elem = float(NCORES * O * K)

    nc = bacc.Bacc("TRN2", target_bir_lowering=False, debug=False,
                   num_devices=NCORES)

    x_ap = nc.dram_tensor("x", [T, K], dt.float32, kind="ExternalInput").ap()
    w_ap = nc.dram_tensor("w", [O, K], dt.float32, kind="ExternalInput").ap()
    out_ap = nc.dram_tensor("out", [T, O], dt.float32, kind="ExternalOutput").ap()

    with tile.TileContext(nc) as tc:
        with (
            tc.tile_pool(name="const", bufs=1) as const_pool,
            tc.tile_pool(name="wres", bufs=1) as wres_pool,
            tc.tile_pool(name="xs", bufs=2) as x_pool,
            tc.tile_pool(name="xqf", bufs=1) as xqf_pool,
            tc.tile_pool(name="xq", bufs=2) as xq_pool,
            tc.tile_pool(name="xqt", bufs=8) as xqt_pool,
            tc.tile_pool(name="sq", bufs=4) as sq_pool,
            tc.tile_pool(name="aa", bufs=3) as a_pool,
            tc.tile_pool(name="sc", bufs=12) as sc_pool,
            tc.tile_pool(name="mmps", bufs=3, space="PSUM") as mm_pool,
            tc.tile_pool(name="wps", bufs=1, space="PSUM") as wps_pool,
            tc.tile_pool(name="dram", bufs=2, space="DRAM") as dram_pool,
        ):
            ident = const_pool.tile([128, 128], dt.bfloat16)
            make_identity(nc, ident[:])
            ones = const_pool.tile([128, 1], dt.float32)
            nc.vector.memset(ones[:], 1.0)

            wqT_cs = [wres_pool.tile([128, KT * 512], dt.bfloat16,
                                     name=f"wqT{c}") for c in range(OC)]
            wqT3_cs = [w[:].rearrange("p (j o) -> p j o", o=512) for w in wqT_cs]
            ws = wres_pool.tile([128, 1], dt.float32)
            halfws = wres_pool.tile([128, 1], dt.float32)
            neghws = wres_pool.tile([128, 1], dt.float32)

            def x_quant(t, head):
                # DMA + per-token scale + exact quantization + xbar transpose
                # for token tile t; returns (xqT, g). Only the tiny g op
                # depends on the collective result ws.
                xt = x_pool.tile([128, K], dt.float32, tag="x", name="x")
                nc.sync.dma_start(xt[:], x_ap[128 * t:128 * (t + 1), :])

                amax = sc_pool.tile([128, 1], dt.float32, tag="amax",
                                    name="amax")
                nc.vector.tensor_reduce(amax[:], xt[:],
                                        axis=mybir.AxisListType.X,
                                        op=Alu.max, apply_absolute_value=True)
                am2 = sc_pool.tile([128, 1], dt.float32, tag="am2", name="am2")
                nc.vector.tensor_scalar_max(am2[:], amax[:], 1e-5)
                rinv = sc_pool.tile([128, 1], dt.float32, tag="rinv",
                                    name="rinv")
                nc.vector.reciprocal(rinv[:], am2[:])
                rs = sc_pool.tile([128, 1], dt.float32, tag="rs", name="rs")
                nc.vector.tensor_scalar_mul(rs[:], rinv[:], float(max_val))
                g = sc_pool.tile([128, 1], dt.float32, tag="g", name="g")
                nc.vector.tensor_tensor(g[:], ws[:], rinv[:], op=Alu.mult)

                # x_q = rint(fl(x * rs)): fp32 product on ACT, then RNE to
                # integer via +C/-C on DVE, cast to exact bf16 integers
                xqf = xqf_pool.tile([128, K], dt.float32, tag="xqf", name="xqf")
                nc.scalar.activation(xqf[:], xt[:],
                                     mybir.ActivationFunctionType.Copy,
                                     scale=rs[:])
                xq = xq_pool.tile([128, K], dt.bfloat16, tag="xq", name="xq")
                rnd_eng = nc.vector if head else nc.gpsimd
                rnd_eng.tensor_scalar(xq[:], xqf[:], C_MAGIC, C_MAGIC,
                                      op0=Alu.add, op1=Alu.subtract)

                # xbar transpose xq -> xqT [128, KT*128] bf16 (k on partitions)
                xqT = xqt_pool.tile([128, KT * 128], dt.bfloat16, tag="xqT",
                                    name="xqT")
                nc.scalar.dma_start_transpose(
                    xqT[:].rearrange("p (j t) -> p j t", t=128), xq[:])
                return xqT, g

            def gemm(c, xqT):
                ps = mm_pool.tile([128, 512], dt.float32, tag=f"mm{c}",
                                  name=f"mm{c}")
                for j in range(KT):
                    nc.tensor.matmul(ps[:], xqT[:, 128 * j:128 * (j + 1)],
                                     wqT3_cs[c][:, j, :],
                                     start=(j == 0), stop=(j == KT - 1))
                return ps

            def gemm2(xqTv):
                # both chunks with a shared stationary tile per j
                pss = [mm_pool.tile([128, 512], dt.float32, tag=f"mm{c}",
                                    name=f"mm{c}") for c in range(OC)]
                for j in range(KT):
                    lhsT = xqTv[:, 128 * j:128 * (j + 1)]
                    for c in range(OC):
                        nc.tensor.matmul(pss[c][:], lhsT, wqT3_cs[c][:, j, :],
                                         start=(j == 0), stop=(j == KT - 1))
                return pss

            def drain(t, c, ps, g):
                # out chunk = (max(psum*g, 0))^2, written as [128, 512]
                a = a_pool.tile([128, 512], dt.float32, tag="a", name="a")
                nc.vector.tensor_scalar(a[:], ps[:], g[:], 0.0,
                                        op0=Alu.mult, op1=Alu.max)
                sq = sq_pool.tile([128, 512], dt.float32, tag="sq", name="sq")
                nc.vector.tensor_tensor(sq[:], a[:], a[:], op=Alu.mult)
                nc.sync.dma_start(
                    out_ap[128 * t:128 * (t + 1), 512 * c:512 * (c + 1)],
                    sq[:])

            # ------------- weight phase (staging pools freed after) -------------
            with (
                tc.tile_pool(name="w32", bufs=1) as w32_pool,
                tc.tile_pool(name="wq", bufs=2) as wq_pool,
            ):
                w32s = [w32_pool.tile([128, K], dt.float32, name=f"w32_{r}")
                        for r in range(OT)]
                wpart = wres_pool.tile([128, OT * KT], dt.float32)

                with tc.high_priority():
                    # pass 1: stream w tiles (kept resident), |w| partial sums
                    for r in range(OT):
                        nc.scalar.dma_start(w32s[r][:],
                                            w_ap[128 * r:128 * (r + 1), :])
                        nc.vector.tensor_reduce(
                            wpart[:, KT * r:KT * (r + 1)],
                            w32s[r][:].rearrange("p (a b) -> p a b", b=128),
                            axis=mybir.AxisListType.X,
                            op=Alu.add, apply_absolute_value=True)
                    wpart1 = wres_pool.tile([128, 1], dt.float32)
                    nc.vector.tensor_reduce(wpart1[:], wpart[:],
                                            axis=mybir.AxisListType.X,
                                            op=Alu.add)
                    # partition sum on the PE: [1,1] = wpart1.T @ ones (fp32)
                    wtot_ps = wps_pool.tile([1, 1], dt.float32, tag="wtot")
                    nc.tensor.matmul(wtot_ps[:], wpart1[:], ones[:],
                                     start=True, stop=True)
                    wtot = wres_pool.tile([1, 1], dt.float32)
                    nc.vector.tensor_copy(wtot[:], wtot_ps[:])

                    # scalar AllReduce across the 8 cores via DRAM bounce bufs
                    cc_in = dram_pool.tile([1, 1], dt.float32)
                    cc_out = dram_pool.tile([1, 1], dt.float32)
                    nc.gpsimd.dma_start(cc_in[:], wtot[:])
                    nc.gpsimd.collective_compute(
                        "AllReduce", Alu.add,
                        replica_groups=[list(range(NCORES))],
                        ins=[cc_in.opt()], outs=[cc_out.opt()])
                    wsum_bc = wres_pool.tile([128, 1], dt.float32)
                    nc.gpsimd.dma_start(wsum_bc[:],
                                        cc_out[:].broadcast_to([128, 1]))

                    ws_inst = nc.vector.tensor_scalar_mul(ws[:], wsum_bc[:],
                                                          1.0 / n_w_elem)
                    nc.vector.tensor_scalar_mul(halfws[:], ws[:], 0.5)
                    nc.vector.tensor_scalar_mul(neghws[:], ws[:], -0.5)

                def w_quant(r):
                    # w_q = (w > 0.5ws) - (w < -0.5ws) via exact strict
                    # compares (0/1 results are exact in bf16), then xbar
                    # transpose into the wqT chunk
                    tp = wq_pool.tile([128, K], dt.bfloat16, tag="wtp", bufs=1)
                    nc.vector.tensor_scalar(tp[:], w32s[r][:], halfws[:], None,
                                            op0=Alu.is_gt)
                    tn = wq_pool.tile([128, K], dt.bfloat16, tag="wtn", bufs=1)
                    nc.vector.tensor_scalar(tn[:], w32s[r][:], neghws[:], None,
                                            op0=Alu.is_lt)
                    wq = wq_pool.tile([128, K], dt.bfloat16, tag="wq")
                    nc.vector.tensor_tensor(wq[:], tp[:], tn[:],
                                            op=Alu.subtract)
                    c, rr = r // 4, r % 4
                    nc.scalar.dma_start_transpose(
                        wqT3_cs[c][:, :, 128 * rr:128 * (rr + 1)], wq[:])

                # chunk 0 of the weights first, at high priority
                with tc.high_priority():
                    for r in range(4):
                        w_quant(r)

                # head of x tiles, processed in the collective's shadow (only
                # each head tile's tiny g op actually waits for ws)
                head_tiles = [x_quant(t, head=True) for t in range(HEAD)]

                # HAM warmup between the collective result and the GEMM start
                warm_ps = wps_pool.tile([128, 128], dt.float32, tag="warm",
                                        bufs=1)
                for i in range(WARMUP_MM):
                    mm = nc.tensor.matmul(warm_ps[:], ident[:], ident[:],
                                          start=True, stop=True)
                    if i == 0:
                        add_dep_helper(mm.ins, ws_inst.ins, sync=True,
                                       reason="HAM warmup after AllReduce")

                # remaining weight chunks
                for r in range(4, OT):
                    w_quant(r)

                # ---------------- main loop over token tiles ----------------
                # emitted inside the weight-phase pool scope for the head part
                deferred = []
                for t in range(HEAD_C0):
                    xqT, g = head_tiles[t]
                    ps0 = gemm(0, xqT)
                    drain(t, 0, ps0, g)
                    deferred.append((t, xqT, g))

            # flush deferred chunk-1.. work, then steady state
            for (t, xqT, g) in deferred:
                for c in range(1, OC):
                    psc = gemm(c, xqT)
                    drain(t, c, psc, g)

            for t in range(HEAD_C0, TT):
                if t < HEAD:
                    xqT, g = head_tiles[t]
                else:
                    xqT, g = x_quant(t, head=False)
                pss = gemm2(xqT[:])
                for c in range(OC):
                    drain(t, c, pss[c], g)

    nc.compile()
    return nc


def _get_nc(T, K, O, max_val):
    key = (T, K, O, max_val)
    if key not in _NC_CACHE:
        _NC_CACHE[key] = _build(T, K, O, max_val)
    return _NC_CACHE[key]


def kernel(x, weight, bits=8):
    global LAST_RESULTS
    x = np.asarray(x, dtype=np.float32)
    weight = np.asarray(weight, dtype=np.float32)
    bits = int(bits)
    max_val = (1 << (bits - 1)) - 1

    lead_shape = x.shape[:-1]
    K = x.shape[-1]
    T = int(np.prod(lead_shape))
    O_total, K_w = weight.shape
    assert K == K_w and O_total % NCORES == 0
    O = O_total // NCORES

    nc = _get_nc(T, K, O, max_val)

    x2 = np.ascontiguousarray(x.reshape(T, K))
    in_maps = [{"x": x2, "w": np.ascontiguousarray(weight[i * O:(i + 1) * O])}
               for i in range(NCORES)]
    res = run_bass_kernel_spmd(nc, in_maps, list(range(NCORES)))
    LAST_RESULTS = res

    out = np.concatenate([res.results[i]["out"] for i in range(NCORES)], axis=1)
    return out.reshape(*lead_shape, O_total)


# revision 18
# speedup vs baseline: 2.8325x; 2.8325x over previous
"""BitLinear (activation int8-quant + ternary weight) + squared-ReLU on 8 Trainium2
NeuronCores.

Sharding: tensor-parallel over weight rows (out_features). Each core receives the
full activation tensor and a 1/8 slice of the weight matrix, computes its slice of
the GEMM + squared ReLU, and the host concatenates the slices.

v2 layout of work (vs the first working version):
  - All transposes (x_q tiles and w_q row-tiles) run on the DMA xbar
    (dma_start_transpose, 2-byte dtype) instead of the PE, so the PE runs the
    bf16 GEMM stream only.
  - w is DMAed once and kept resident in SBUF as f32; the quantization pass
    reads it from SBUF after the w_scale AllReduce (no second HBM pass).
  - The w_scale chain (|w| partial sums -> partition sum via a tiny fp32
    matmul -> scalar AllReduce -> thresholds -> chunk-0 quantize+transpose) is
    emitted under tc.high_priority() so the scheduler runs it as early as the
    data allows.
  - Two HWDGE rings split the DMA traffic: sync carries x in + out writes,
    scalar carries w in + all xbar transposes.
  - Engine balance: gpsimd does the per-token amax for steady-state tiles
    (DVE covers the head tiles while gpsimd is blocked on the collective),
    ACT does the x*(127/amax) scale, DVE does the exact rounding, the scale
    scalars, the w compares, and the output drain.
  - Output drain is relu(g*psum)^2 computed as a = max(psum*g, 0) (one
    tensor_scalar) then a*a (one tensor_tensor), written per 512-wide chunk.
  - The first HEAD_C0 tiles' chunk-0 GEMMs are emitted before any chunk-1
    work so the PE can start as soon as the first half of the weights is
    quantized.

Math notes (unchanged):
  - x_q = round(x * 127/scale), scale = clip(amax_row(|x|), 1e-5). Values are
    integers in [-127, 127] -> exact in bf16.
  - w_q in {-1, 0, 1} = (w > 0.5*ws) - (w < -0.5*ws) via exact fp32 strict
    compares; ws = mean(|W|) over the full weight (AllReduce of per-core sums).
  - bf16 GEMM with fp32 PSUM accumulation is exact (integer products, partial
    sums < 2^24).
  - Rounding uses the +1.5*2^23 magic-constant trick after the product is
    rounded to fp32 (same double-rounding as the reference).
"""

import sys

if "/opt/trn_rl_repo" not in sys.path:
    sys.path.insert(0, "/opt/trn_rl_repo")

import numpy as np

import concourse.bacc as bacc
import concourse.mybir as mybir
import concourse.tile as tile
from concourse.bass_utils import run_bass_kernel_spmd
from concourse.masks import make_identity
from concourse.tile import add_dep_helper

dt = mybir.dt
Alu = mybir.AluOpType
NCORES = 8
C_MAGIC = 1.5 * 2**23  # fp32 round-to-nearest-even forcing constant
HEAD = 8               # x tiles pre-processed during the weight phase
HEAD_C0 = 6            # head tiles whose chunk-0 GEMM runs before chunk 1 exists
WARMUP_MM = 40         # HAM warmup matmuls between the collective and the GEMM

# Stash of the most recent BassKernelResults (test harness reads exec_time_ns).
LAST_RESULTS = None

_NC_CACHE = {}


def _build(T, K, O, max_val):
    """Build + compile the per-core Bass module.

    Per-core tensors: x [T, K] f32 (replicated), w [O, K] f32 (this core's rows),
    out [T, O] f32.
    """
    assert T % 128 == 0 and K % 128 == 0 and O % 512 == 0
    TT = T // 128     # token tiles
    KT = K // 128     # contraction tiles
    OC = O // 512     # psum-width output chunks per core
    OT = O // 128     # weight row tiles
    n_w_elem = float(NCORES * O * K)

    nc = bacc.Bacc("TRN2", target_bir_lowering=False, debug=False,
                   num_devices=NCORES)

    x_ap = nc.dram_tensor("x", [T, K], dt.float32, kind="ExternalInput").ap()
    w_ap = nc.dram_tensor("w", [O, K], dt.float32, kind="ExternalInput").ap()
    out_ap = nc.dram_tensor("out", [T, O], dt.float32, kind="ExternalOutput").ap()

    with tile.TileContext(nc) as tc:
        with (
            tc.tile_pool(name="const", bufs=1) as const_pool,
            tc.tile_pool(name="wres", bufs=1) as wres_pool,
            tc.tile_pool(name="xs", bufs=2) as x_pool,
            tc.tile_pool(name="xqf", bufs=1) as xqf_pool,
            tc.tile_pool(name="xq", bufs=3) as xq_pool,
            tc.tile_pool(name="xqt", bufs=8) as xqt_pool,
            tc.tile_pool(name="sq", bufs=4) as sq_pool,
            tc.tile_pool(name="aa", bufs=3) as a_pool,
            tc.tile_pool(name="sc", bufs=12) as sc_pool,
            tc.tile_pool(name="mmps", bufs=3, space="PSUM") as mm_pool,
            tc.tile_pool(name="wps", bufs=1, space="PSUM") as wps_pool,
            tc.tile_pool(name="dram", bufs=2, space="DRAM") as dram_pool,
        ):
            ident = const_pool.tile([128, 128], dt.bfloat16)
            make_identity(nc, ident[:])
            ones = const_pool.tile([128, 1], dt.float32)
            nc.vector.memset(ones[:], 1.0)

            wqT_cs = [wres_pool.tile([128, KT * 512], dt.bfloat16,
                                     name=f"wqT{c}") for c in range(OC)]
            wqT3_cs = [w[:].rearrange("p (j o) -> p j o", o=512) for w in wqT_cs]
            ws = wres_pool.tile([128, 1], dt.float32)
            halfws = wres_pool.tile([128, 1], dt.float32)
            neghws = wres_pool.tile([128, 1], dt.float32)

            def x_quant(t, head, order_after=None):
                # DMA + per-token scale + exact quantization + xbar transpose
                # for token tile t; returns (xqT, g). Only the tiny g op
                # depends on the collective result ws. order_after adds a
                # scheduling-only edge so steady-tile DVE work cannot crowd
                # out the post-AllReduce weight-quantization chain.
                xt = x_pool.tile([128, K], dt.float32, tag="x", name="x")
                nc.sync.dma_start(xt[:], x_ap[128 * t:128 * (t + 1), :])

                amax = sc_pool.tile([128, 1], dt.float32, tag="amax",
                                    name="amax")
                am_inst = nc.vector.tensor_reduce(amax[:], xt[:],
                                                  axis=mybir.AxisListType.X,
                                                  op=Alu.max,
                                                  apply_absolute_value=True)
                if order_after is not None:
                    add_dep_helper(am_inst.ins, order_after.ins, sync=False,
                                   reason="steady x work after wq chain")
                am2 = sc_pool.tile([128, 1], dt.float32, tag="am2", name="am2")
                nc.vector.tensor_scalar_max(am2[:], amax[:], 1e-5)
                rinv = sc_pool.tile([128, 1], dt.float32, tag="rinv",
                                    name="rinv")
                nc.vector.reciprocal(rinv[:], am2[:])
                rs = sc_pool.tile([128, 1], dt.float32, tag="rs", name="rs")
                nc.vector.tensor_scalar_mul(rs[:], rinv[:], float(max_val))
                g = sc_pool.tile([128, 1], dt.float32, tag="g", name="g")
                nc.vector.tensor_tensor(g[:], ws[:], rinv[:], op=Alu.mult)

                # x_q = rint(fl(x * rs)): fp32 product on ACT, then RNE to
                # integer via +C/-C on DVE, cast to exact bf16 integers
                xqf = xqf_pool.tile([128, K], dt.float32, tag="xqf", name="xqf")
                nc.scalar.activation(xqf[:], xt[:],
                                     mybir.ActivationFunctionType.Copy,
                                     scale=rs[:])
                xq = xq_pool.tile([128, K], dt.bfloat16, tag="xq", name="xq")
                nc.vector.tensor_scalar(xq[:], xqf[:], C_MAGIC, C_MAGIC,
                                        op0=Alu.add, op1=Alu.subtract)

                # xbar transpose xq -> xqT [128, KT*128] bf16 (k on partitions)
                xqT = xqt_pool.tile([128, KT * 128], dt.bfloat16, tag="xqT",
                                    name="xqT")
                nc.scalar.dma_start_transpose(
                    xqT[:].rearrange("p (j t) -> p j t", t=128), xq[:])
                return xqT, g

            def gemm(c, xqT):
                ps = mm_pool.tile([128, 512], dt.float32, tag=f"mm{c}",
                                  name=f"mm{c}")
                for j in range(KT):
                    nc.tensor.matmul(ps[:], xqT[:, 128 * j:128 * (j + 1)],
                                     wqT3_cs[c][:, j, :],
                                     start=(j == 0), stop=(j == KT - 1))
                return ps

            def gemm2(xqTv):
                # chunk-major: all 16 k-steps into one psum bank, then the
                # next bank (alternating banks per-MM makes the PE micro-idle)
                pss = []
                for c in range(OC):
                    ps = mm_pool.tile([128, 512], dt.float32, tag=f"mm{c}",
                                      name=f"mm{c}")
                    for j in range(KT):
                        nc.tensor.matmul(ps[:], xqTv[:, 128 * j:128 * (j + 1)],
                                         wqT3_cs[c][:, j, :],
                                         start=(j == 0), stop=(j == KT - 1))
                    pss.append(ps)
                return pss

            def drain(t, c, ps, g):
                # out chunk = (max(psum*g, 0))^2, written as [128, 512]
                a = a_pool.tile([128, 512], dt.float32, tag="a", name="a")
                nc.vector.tensor_scalar(a[:], ps[:], g[:], 0.0,
                                        op0=Alu.mult, op1=Alu.max)
                sq = sq_pool.tile([128, 512], dt.float32, tag="sq", name="sq")
                nc.vector.tensor_tensor(sq[:], a[:], a[:], op=Alu.mult)
                nc.sync.dma_start(
                    out_ap[128 * t:128 * (t + 1), 512 * c:512 * (c + 1)],
                    sq[:])

            # ------------- weight phase (staging pools freed after) -------------
            with (
                tc.tile_pool(name="w32", bufs=1) as w32_pool,
                tc.tile_pool(name="wq", bufs=2) as wq_pool,
            ):
                w32s = [w32_pool.tile([128, K], dt.float32, name=f"w32_{r}")
                        for r in range(OT)]
                wpart = wres_pool.tile([128, OT], dt.float32)

                with tc.high_priority():
                    # pass 1: stream w tiles (kept resident), |w| partial sums
                    for r in range(OT):
                        nc.scalar.dma_start(w32s[r][:],
                                            w_ap[128 * r:128 * (r + 1), :])
                        nc.vector.tensor_reduce(
                            wpart[:, r:r + 1], w32s[r][:],
                            axis=mybir.AxisListType.X,
                            op=Alu.add, apply_absolute_value=True)
                    wpart1 = wres_pool.tile([128, 1], dt.float32)
                    nc.vector.tensor_reduce(wpart1[:], wpart[:],
                                            axis=mybir.AxisListType.X,
                                            op=Alu.add)
                    # partition sum on the PE: [1,1] = wpart1.T @ ones (fp32)
                    wtot_ps = wps_pool.tile([1, 1], dt.float32, tag="wtot")
                    nc.tensor.matmul(wtot_ps[:], wpart1[:], ones[:],
                                     start=True, stop=True)
                    wtot = wres_pool.tile([1, 1], dt.float32)
                    nc.vector.tensor_copy(wtot[:], wtot_ps[:])

                    # scalar AllGather across the 8 cores via DRAM bounce bufs
                    # (cheaper floor than AllReduce for 4 bytes); each core
                    # sums the 8 gathered values locally. Bounce DMAs ride the
                    # fast HWDGE rings; gpsimd only triggers the collective.
                    cc_in = dram_pool.tile([1, 1], dt.float32)
                    cc_out = dram_pool.tile([NCORES, 1], dt.float32)
                    nc.sync.dma_start(cc_in[:], wtot[:])
                    nc.gpsimd.collective_compute(
                        "AllGather", Alu.bypass,
                        replica_groups=[list(range(NCORES))],
                        ins=[cc_in.opt()], outs=[cc_out.opt()])
                    wsum8 = wres_pool.tile([128, NCORES], dt.float32)
                    nc.sync.dma_start(
                        wsum8[:],
                        cc_out[:].rearrange("a b -> b a").broadcast_to(
                            [128, NCORES]))
                    wsum_bc = wres_pool.tile([128, 1], dt.float32)
                    nc.vector.tensor_reduce(wsum_bc[:], wsum8[:],
                                            axis=mybir.AxisListType.X,
                                            op=Alu.add)

                    ws_inst = nc.vector.tensor_scalar_mul(ws[:], wsum_bc[:],
                                                          1.0 / n_w_elem)
                    nc.vector.tensor_scalar_mul(halfws[:], ws[:], 0.5)
                    nc.vector.tensor_scalar_mul(neghws[:], ws[:], -0.5)

                def w_quant(r):
                    # w_q = (w > 0.5ws) - (w < -0.5ws) via exact strict
                    # compares (0/1 results are exact in bf16), then xbar
                    # transpose into the wqT chunk
                    tp = wq_pool.tile([128, K], dt.bfloat16, tag="wtp", bufs=1)
                    nc.vector.tensor_scalar(tp[:], w32s[r][:], halfws[:], None,
                                            op0=Alu.is_gt)
                    tn = wq_pool.tile([128, K], dt.bfloat16, tag="wtn", bufs=1)
                    nc.vector.tensor_scalar(tn[:], w32s[r][:], neghws[:], None,
                                            op0=Alu.is_lt)
                    wq = wq_pool.tile([128, K], dt.bfloat16, tag="wq")
                    tt = nc.vector.tensor_tensor(wq[:], tp[:], tn[:],
                                                 op=Alu.subtract)
                    c, rr = r // 4, r % 4
                    xb = nc.scalar.dma_start_transpose(
                        wqT3_cs[c][:, :, 128 * rr:128 * (rr + 1)], wq[:])
                    return tt, xb

                # chunk 0 of the weights first, at high priority
                with tc.high_priority():
                    wq_insts = [w_quant(r) for r in range(4)]

                # head of x tiles, processed in the collective's shadow (only
                # each head tile's tiny g op actually waits for ws)
                head_tiles = [x_quant(t, head=True) for t in range(HEAD)]

                # remaining weight chunks
                with tc.high_priority():
                    wq_insts += [w_quant(r) for r in range(4, OT)]
                wq_last = wq_insts[-1][0]

                # ---------------- main loop over token tiles ----------------
                # emitted inside the weight-phase pool scope for the head part
                deferred = []
                for t in range(HEAD_C0):
                    xqT, g = head_tiles[t]
                    ps0 = gemm(0, xqT)
                    drain(t, 0, ps0, g)
                    deferred.append((t, xqT, g))

            # flush deferred chunk-1.. work, then steady state
            for (t, xqT, g) in deferred:
                for c in range(1, OC):
                    psc = gemm(c, xqT)
                    drain(t, c, psc, g)

            for t in range(HEAD_C0, TT):
                if t < HEAD:
                    xqT, g = head_tiles[t]
                else:
                    xqT, g = x_quant(t, head=False, order_after=wq_last)
                pss = gemm2(xqT[:])
                for c in range(OC):
                    drain(t, c, pss[c], g)

    nc.compile()
    return nc


def _get_nc(T, K, O, max_val):
    key = (T, K, O, max_val)
    if key not in _NC_CACHE:
        _NC_CACHE[key] = _build(T, K, O, max_val)
    return _NC_CACHE[key]


def kernel(x, weight, bits=8):
    global LAST_RESULTS
    x = np.asarray(x, dtype=np.float32)
    weight = np.asarray(weight, dtype=np.float32)
    bits = int(bits)
    max_val = (1 << (bits - 1)) - 1

    lead_shape = x.shape[:-1]
    K = x.shape[-1]
    T = int(np.prod(lead_shape))
    O_total, K_w = weight.shape
    assert K == K_w and O_total % NCORES == 0
    O = O_total // NCORES

    nc = _get_nc(T, K, O, max_val)

    x2 = np.ascontiguousarray(x.reshape(T, K))
    in_maps = [{"x": x2, "w": np.ascontiguousarray(weight[i * O:(i + 1) * O])}
               for i in range(NCORES)]
    res = run_bass_kernel_spmd(nc, in_maps, list(range(NCORES)))
    LAST_RESULTS = res

    out = np.concatenate([res.results[i]["out"] for i in range(NCORES)], axis=1)
    return out.reshape(*lead_shape, O_total)


# revision 24
# speedup vs baseline: 3.2890x; 1.1612x over previous
"""BitLinear (activation int8-quant + ternary weight) + squared-ReLU on 8 Trainium2
NeuronCores.

Sharding: tensor-parallel over weight rows (out_features). Each core receives the
full activation tensor and a 1/8 slice of the weight matrix, computes its slice of
the GEMM + squared ReLU, and the host concatenates the slices.

v2 layout of work (vs the first working version):
  - All transposes (x_q tiles and w_q row-tiles) run on the DMA xbar
    (dma_start_transpose, 2-byte dtype) instead of the PE, so the PE runs the
    bf16 GEMM stream only.
  - w is DMAed once and kept resident in SBUF as f32; the quantization pass
    reads it from SBUF after the w_scale AllReduce (no second HBM pass).
  - The w_scale chain (|w| partial sums -> partition sum via a tiny fp32
    matmul -> scalar AllReduce -> thresholds -> chunk-0 quantize+transpose) is
    emitted under tc.high_priority() so the scheduler runs it as early as the
    data allows.
  - Two HWDGE rings split the DMA traffic: sync carries x in + out writes,
    scalar carries w in + all xbar transposes.
  - Engine balance: gpsimd does the per-token amax for steady-state tiles
    (DVE covers the head tiles while gpsimd is blocked on the collective),
    ACT does the x*(127/amax) scale, DVE does the exact rounding, the scale
    scalars, the w compares, and the output drain.
  - Output drain is relu(g*psum)^2 computed as a = max(psum*g, 0) (one
    tensor_scalar) then a*a (one tensor_tensor), written per 512-wide chunk.
  - The first HEAD_C0 tiles' chunk-0 GEMMs are emitted before any chunk-1
    work so the PE can start as soon as the first half of the weights is
    quantized.

Math notes (unchanged):
  - x_q = round(x * 127/scale), scale = clip(amax_row(|x|), 1e-5). Values are
    integers in [-127, 127] -> exact in bf16.
  - w_q in {-1, 0, 1} = (w > 0.5*ws) - (w < -0.5*ws) via exact fp32 strict
    compares; ws = mean(|W|) over the full weight (AllReduce of per-core sums).
  - bf16 GEMM with fp32 PSUM accumulation is exact (integer products, partial
    sums < 2^24).
  - Rounding uses the +1.5*2^23 magic-constant trick after the product is
    rounded to fp32 (same double-rounding as the reference).
"""

import sys

if "/opt/trn_rl_repo" not in sys.path:
    sys.path.insert(0, "/opt/trn_rl_repo")

import numpy as np

import concourse.bacc as bacc
import concourse.bass_isa as bass_isa
import concourse.mybir as mybir
import concourse.tile as tile
from concourse.bass_utils import run_bass_kernel_spmd
from concourse.masks import make_identity
from concourse.tile import add_dep_helper

dt = mybir.dt
Alu = mybir.AluOpType
NCORES = 8
C_MAGIC = 1.5 * 2**23  # fp32 round-to-nearest-even forcing constant
HEAD = 8               # x tiles pre-processed during the weight phase
HEAD_C0 = 6            # head tiles whose chunk-0 GEMM runs before chunk 1 exists
WARMUP_MM = 40         # HAM warmup matmuls between the collective and the GEMM

# Stash of the most recent BassKernelResults (test harness reads exec_time_ns).
LAST_RESULTS = None

_NC_CACHE = {}


def _build(T, K, O, max_val):
    """Build + compile the per-core Bass module.

    Per-core tensors: x [T, K] f32 (replicated), w [O, K] f32 (this core's rows),
    out [T, O] f32.
    """
    assert T % 128 == 0 and K % 128 == 0 and O % 512 == 0
    TT = T // 128     # token tiles
    KT = K // 128     # contraction tiles
    OC = O // 512     # psum-width output chunks per core
    OT = O // 128     # weight row tiles
    n_w_elem = float(NCORES * O * K)

    nc = bacc.Bacc("TRN2", target_bir_lowering=False, debug=False,
                   num_devices=NCORES)

    x_ap = nc.dram_tensor("x", [T, K], dt.float32, kind="ExternalInput").ap()
    w_ap = nc.dram_tensor("w", [O, K], dt.float32, kind="ExternalInput").ap()
    out_ap = nc.dram_tensor("out", [T, O], dt.float32, kind="ExternalOutput").ap()

    with tile.TileContext(nc) as tc:
        with (
            tc.tile_pool(name="const", bufs=1) as const_pool,
            tc.tile_pool(name="wres", bufs=1) as wres_pool,
            tc.tile_pool(name="xs", bufs=2) as x_pool,
            tc.tile_pool(name="xqf", bufs=1) as xqf_pool,
            tc.tile_pool(name="xq", bufs=3) as xq_pool,
            tc.tile_pool(name="xqt", bufs=8) as xqt_pool,
            tc.tile_pool(name="sq", bufs=4) as sq_pool,
            tc.tile_pool(name="aa", bufs=3) as a_pool,
            tc.tile_pool(name="sc", bufs=12) as sc_pool,
            tc.tile_pool(name="mmps", bufs=3, space="PSUM") as mm_pool,
            tc.tile_pool(name="tps", bufs=2, space="PSUM") as tps_pool,
            tc.tile_pool(name="dram", bufs=2, space="DRAM") as dram_pool,
        ):
            ident = const_pool.tile([128, 128], dt.bfloat16)
            make_identity(nc, ident[:])

            wqT_cs = [wres_pool.tile([128, KT * 512], dt.bfloat16,
                                     name=f"wqT{c}") for c in range(OC)]
            wqT3_cs = [w[:].rearrange("p (j o) -> p j o", o=512) for w in wqT_cs]
            ws = wres_pool.tile([128, 1], dt.float32)
            halfws = wres_pool.tile([128, 1], dt.float32)
            neghws = wres_pool.tile([128, 1], dt.float32)

            def x_quant(t, head, order_after=None):
                # DMA + per-token scale + exact quantization + xbar transpose
                # for token tile t; returns (xqT, g). Only the tiny g op
                # depends on the collective result ws. order_after adds a
                # scheduling-only edge so steady-tile DVE work cannot crowd
                # out the post-AllReduce weight-quantization chain.
                xt = x_pool.tile([128, K], dt.float32, tag="x", name="x")
                nc.sync.dma_start(xt[:], x_ap[128 * t:128 * (t + 1), :])

                amax = sc_pool.tile([128, 1], dt.float32, tag="amax",
                                    name="amax")
                am_inst = nc.vector.tensor_reduce(amax[:], xt[:],
                                                  axis=mybir.AxisListType.X,
                                                  op=Alu.max,
                                                  apply_absolute_value=True)
                if order_after is not None:
                    add_dep_helper(am_inst.ins, order_after.ins, sync=False,
                                   reason="steady x work after wq chain")
                am2 = sc_pool.tile([128, 1], dt.float32, tag="am2", name="am2")
                nc.vector.tensor_scalar_max(am2[:], amax[:], 1e-5)
                rinv = sc_pool.tile([128, 1], dt.float32, tag="rinv",
                                    name="rinv")
                nc.vector.reciprocal(rinv[:], am2[:])
                rs = sc_pool.tile([128, 1], dt.float32, tag="rs", name="rs")
                nc.vector.tensor_scalar_mul(rs[:], rinv[:], float(max_val))
                g = sc_pool.tile([128, 1], dt.float32, tag="g", name="g")
                nc.vector.tensor_tensor(g[:], ws[:], rinv[:], op=Alu.mult)

                # x_q = rint(fl(x * rs)): fp32 product on ACT, then RNE to
                # integer via +C/-C on DVE, cast to exact bf16 integers
                xqf = xqf_pool.tile([128, K], dt.float32, tag="xqf", name="xqf")
                nc.scalar.activation(xqf[:], xt[:],
                                     mybir.ActivationFunctionType.Copy,
                                     scale=rs[:])
                xq = xq_pool.tile([128, K], dt.bfloat16, tag="xq", name="xq")
                nc.vector.tensor_scalar(xq[:], xqf[:], C_MAGIC, C_MAGIC,
                                        op0=Alu.add, op1=Alu.subtract)

                # PE transpose xq -> xqT [128, KT*128] bf16 (k on partitions);
                # the PE interleaves these with GEMM matmuls without breaking
                # the stream (a DMA-xbar transpose here stalls the PE's SBUF
                # reads and is serialized against the collective)
                xqT = xqt_pool.tile([128, KT * 128], dt.bfloat16, tag="xqT",
                                    name="xqT")
                half = KT // 2
                for hh in range(2):
                    ps = tps_pool.tile([128, half * 128], dt.bfloat16,
                                       tag="tps", name="tps")
                    for q in range(half):
                        j = hh * half + q
                        nc.tensor.transpose(
                            ps[:, 128 * q:128 * (q + 1)],
                            xq[:, 128 * j:128 * (j + 1)], ident[:])
                    dst = xqT[:, 128 * half * hh:128 * half * (hh + 1)]
                    if hh == 0:
                        nc.scalar.copy(dst, ps[:])
                    else:
                        nc.vector.tensor_copy(dst, ps[:])
                return xqT, g

            def gemm(c, xqT):
                ps = mm_pool.tile([128, 512], dt.float32, tag=f"mm{c}",
                                  name=f"mm{c}")
                for j in range(KT):
                    nc.tensor.matmul(ps[:], xqT[:, 128 * j:128 * (j + 1)],
                                     wqT3_cs[c][:, j, :],
                                     start=(j == 0), stop=(j == KT - 1))
                return ps

            def gemm2(xqTv):
                # chunk-major: all 16 k-steps into one psum bank, then the
                # next bank (alternating banks per-MM makes the PE micro-idle)
                pss = []
                for c in range(OC):
                    ps = mm_pool.tile([128, 512], dt.float32, tag=f"mm{c}",
                                      name=f"mm{c}")
                    for j in range(KT):
                        nc.tensor.matmul(ps[:], xqTv[:, 128 * j:128 * (j + 1)],
                                         wqT3_cs[c][:, j, :],
                                         start=(j == 0), stop=(j == KT - 1))
                    pss.append(ps)
                return pss

            def drain(t, c, ps, g):
                # out chunk = (max(psum*g, 0))^2, written as [128, 512]
                a = a_pool.tile([128, 512], dt.float32, tag="a", name="a")
                nc.vector.tensor_scalar(a[:], ps[:], g[:], 0.0,
                                        op0=Alu.mult, op1=Alu.max)
                sq = sq_pool.tile([128, 512], dt.float32, tag="sq", name="sq")
                nc.vector.tensor_tensor(sq[:], a[:], a[:], op=Alu.mult)
                nc.sync.dma_start(
                    out_ap[128 * t:128 * (t + 1), 512 * c:512 * (c + 1)],
                    sq[:])

            # ------------- weight phase (staging pools freed after) -------------
            with (
                tc.tile_pool(name="w32", bufs=1) as w32_pool,
                tc.tile_pool(name="wq", bufs=2) as wq_pool,
            ):
                w32s = [w32_pool.tile([128, K], dt.float32, name=f"w32_{r}")
                        for r in range(OT)]
                wpart = wres_pool.tile([128, OT], dt.float32)

                with tc.high_priority():
                    # pass 1: stream w tiles (kept resident), |w| partial sums
                    for r in range(OT):
                        nc.scalar.dma_start(w32s[r][:],
                                            w_ap[128 * r:128 * (r + 1), :])
                        nc.vector.tensor_reduce(
                            wpart[:, r:r + 1], w32s[r][:],
                            axis=mybir.AxisListType.X,
                            op=Alu.add, apply_absolute_value=True)
                    wpart1 = wres_pool.tile([128, 1], dt.float32)
                    nc.vector.tensor_reduce(wpart1[:], wpart[:],
                                            axis=mybir.AxisListType.X,
                                            op=Alu.add)
                    wtot = wres_pool.tile([128, 1], dt.float32)
                    nc.gpsimd.partition_all_reduce(
                        wtot[:], wpart1[:], channels=128,
                        reduce_op=bass_isa.ReduceOp.add)

                    # scalar AllGather across the 8 cores via DRAM bounce bufs
                    # (cheaper floor than AllReduce for 4 bytes); each core
                    # sums the 8 gathered values locally. Bounce DMAs ride the
                    # fast HWDGE rings; gpsimd only triggers the collective.
                    cc_in = dram_pool.tile([1, 1], dt.float32)
                    cc_out = dram_pool.tile([NCORES, 1], dt.float32)
                    nc.sync.dma_start(cc_in[:], wtot[0:1, 0:1])
                    nc.gpsimd.collective_compute(
                        "AllGather", Alu.bypass,
                        replica_groups=[list(range(NCORES))],
                        ins=[cc_in.opt()], outs=[cc_out.opt()])
                    wsum8 = wres_pool.tile([128, NCORES], dt.float32)
                    nc.sync.dma_start(
                        wsum8[:],
                        cc_out[:].rearrange("a b -> b a").broadcast_to(
                            [128, NCORES]))
                    wsum_bc = wres_pool.tile([128, 1], dt.float32)
                    nc.vector.tensor_reduce(wsum_bc[:], wsum8[:],
                                            axis=mybir.AxisListType.X,
                                            op=Alu.add)

                    ws_inst = nc.vector.tensor_scalar_mul(ws[:], wsum_bc[:],
                                                          1.0 / n_w_elem)
                    nc.vector.tensor_scalar_mul(halfws[:], ws[:], 0.5)
                    nc.vector.tensor_scalar_mul(neghws[:], ws[:], -0.5)

                def w_quant(r):
                    # w_q = (w > 0.5ws) - (w < -0.5ws) via exact strict
                    # compares (0/1 results are exact in bf16), then xbar
                    # transpose into the wqT chunk
                    tp = wq_pool.tile([128, K], dt.bfloat16, tag="wtp", bufs=1)
                    nc.vector.tensor_scalar(tp[:], w32s[r][:], halfws[:], None,
                                            op0=Alu.is_gt)
                    tn = wq_pool.tile([128, K], dt.bfloat16, tag="wtn", bufs=1)
                    nc.vector.tensor_scalar(tn[:], w32s[r][:], neghws[:], None,
                                            op0=Alu.is_lt)
                    wq = wq_pool.tile([128, K], dt.bfloat16, tag="wq")
                    tt = nc.vector.tensor_tensor(wq[:], tp[:], tn[:],
                                                 op=Alu.subtract)
                    c, rr = r // 4, r % 4
                    xb = nc.scalar.dma_start_transpose(
                        wqT3_cs[c][:, :, 128 * rr:128 * (rr + 1)], wq[:])
                    return tt, xb

                # chunk 0 of the weights first, at high priority
                with tc.high_priority():
                    wq_insts = [w_quant(r) for r in range(4)]

                # head of x tiles, processed in the collective's shadow (only
                # each head tile's tiny g op actually waits for ws)
                head_tiles = [x_quant(t, head=True) for t in range(HEAD)]

                # remaining weight chunks
                with tc.high_priority():
                    wq_insts += [w_quant(r) for r in range(4, OT)]
                wq_last = wq_insts[-1][0]

                # ---------------- main loop over token tiles ----------------
                # emitted inside the weight-phase pool scope for the head part
                deferred = []
                for t in range(HEAD_C0):
                    xqT, g = head_tiles[t]
                    ps0 = gemm(0, xqT)
                    drain(t, 0, ps0, g)
                    deferred.append((t, xqT, g))

            # flush deferred chunk-1.. work, then steady state
            for (t, xqT, g) in deferred:
                for c in range(1, OC):
                    psc = gemm(c, xqT)
                    drain(t, c, psc, g)

            for t in range(HEAD_C0, TT):
                if t < HEAD:
                    xqT, g = head_tiles[t]
                else:
                    xqT, g = x_quant(t, head=False, order_after=wq_last)
                pss = gemm2(xqT[:])
                for c in range(OC):
                    drain(t, c, pss[c], g)

    nc.compile()
    return nc


def _get_nc(T, K, O, max_val):
    key = (T, K, O, max_val)
    if key not in _NC_CACHE:
        _NC_CACHE[key] = _build(T, K, O, max_val)
    return _NC_CACHE[key]


def kernel(x, weight, bits=8):
    global LAST_RESULTS
    x = np.asarray(x, dtype=np.float32)
    weight = np.asarray(weight, dtype=np.float32)
    bits = int(bits)
    max_val = (1 << (bits - 1)) - 1

    lead_shape = x.shape[:-1]
    K = x.shape[-1]
    T = int(np.prod(lead_shape))
    O_total, K_w = weight.shape
    assert K == K_w and O_total % NCORES == 0
    O = O_total // NCORES

    nc = _get_nc(T, K, O, max_val)

    x2 = np.ascontiguousarray(x.reshape(T, K))
    in_maps = [{"x": x2, "w": np.ascontiguousarray(weight[i * O:(i + 1) * O])}
               for i in range(NCORES)]
    res = run_bass_kernel_spmd(nc, in_maps, list(range(NCORES)))
    LAST_RESULTS = res

    out = np.concatenate([res.results[i]["out"] for i in range(NCORES)], axis=1)
    return out.reshape(*lead_shape, O_total)


# revision 30
# speedup vs baseline: 3.3604x; 1.0217x over previous
"""BitLinear (activation int8-quant + ternary weight) + squared-ReLU on 8 Trainium2
NeuronCores.

Sharding: tensor-parallel over weight rows (out_features). Each core receives the
full activation tensor and a 1/8 slice of the weight matrix, computes its slice of
the GEMM + squared ReLU, and the host concatenates the slices.

v2 layout of work (vs the first working version):
  - All transposes (x_q tiles and w_q row-tiles) run on the DMA xbar
    (dma_start_transpose, 2-byte dtype) instead of the PE, so the PE runs the
    bf16 GEMM stream only.
  - w is DMAed once and kept resident in SBUF as f32; the quantization pass
    reads it from SBUF after the w_scale AllReduce (no second HBM pass).
  - The w_scale chain (|w| partial sums -> partition sum via a tiny fp32
    matmul -> scalar AllReduce -> thresholds -> chunk-0 quantize+transpose) is
    emitted under tc.high_priority() so the scheduler runs it as early as the
    data allows.
  - Two HWDGE rings split the DMA traffic: sync carries x in + out writes,
    scalar carries w in + all xbar transposes.
  - Engine balance: gpsimd does the per-token amax for steady-state tiles
    (DVE covers the head tiles while gpsimd is blocked on the collective),
    ACT does the x*(127/amax) scale, DVE does the exact rounding, the scale
    scalars, the w compares, and the output drain.
  - Output drain is relu(g*psum)^2 computed as a = max(psum*g, 0) (one
    tensor_scalar) then a*a (one tensor_tensor), written per 512-wide chunk.
  - The first HEAD_C0 tiles' chunk-0 GEMMs are emitted before any chunk-1
    work so the PE can start as soon as the first half of the weights is
    quantized.

Math notes (unchanged):
  - x_q = round(x * 127/scale), scale = clip(amax_row(|x|), 1e-5). Values are
    integers in [-127, 127] -> exact in bf16.
  - w_q in {-1, 0, 1} = (w > 0.5*ws) - (w < -0.5*ws) via exact fp32 strict
    compares; ws = mean(|W|) over the full weight (AllReduce of per-core sums).
  - bf16 GEMM with fp32 PSUM accumulation is exact (integer products, partial
    sums < 2^24).
  - Rounding uses the +1.5*2^23 magic-constant trick after the product is
    rounded to fp32 (same double-rounding as the reference).
"""

import sys

if "/opt/trn_rl_repo" not in sys.path:
    sys.path.insert(0, "/opt/trn_rl_repo")

import numpy as np

import concourse.bacc as bacc
import concourse.bass_isa as bass_isa
import concourse.mybir as mybir
import concourse.tile as tile
from concourse.bass_utils import run_bass_kernel_spmd
from concourse.masks import make_identity
from concourse.tile import add_dep_helper

dt = mybir.dt
Alu = mybir.AluOpType
NCORES = 8
C_MAGIC = 1.5 * 2**23  # fp32 round-to-nearest-even forcing constant
HEAD = 8               # x tiles pre-processed during the weight phase
HEAD_C0 = 6            # head tiles whose chunk-0 GEMM runs before chunk 1 exists
WARMUP_MM = 40         # HAM warmup matmuls between the collective and the GEMM

# Stash of the most recent BassKernelResults (test harness reads exec_time_ns).
LAST_RESULTS = None

_NC_CACHE = {}


def _build(T, K, O, max_val):
    """Build + compile the per-core Bass module.

    Per-core tensors: x [T, K] f32 (replicated), w [O, K] f32 (this core's rows),
    out [T, O] f32.
    """
    assert T % 128 == 0 and K % 128 == 0 and O % 512 == 0
    TT = T // 128     # token tiles
    KT = K // 128     # contraction tiles
    OC = O // 512     # psum-width output chunks per core
    OT = O // 128     # weight row tiles
    n_w_elem = float(NCORES * O * K)

    nc = bacc.Bacc("TRN2", target_bir_lowering=False, debug=False,
                   num_devices=NCORES)

    x_ap = nc.dram_tensor("x", [T, K], dt.float32, kind="ExternalInput").ap()
    w_ap = nc.dram_tensor("w", [O, K], dt.float32, kind="ExternalInput").ap()
    out_ap = nc.dram_tensor("out", [T, O], dt.float32, kind="ExternalOutput").ap()

    with tile.TileContext(nc) as tc:
        with (
            tc.tile_pool(name="const", bufs=1) as const_pool,
            tc.tile_pool(name="wres", bufs=1) as wres_pool,
            tc.tile_pool(name="xs", bufs=2) as x_pool,
            tc.tile_pool(name="xqf", bufs=1) as xqf_pool,
            tc.tile_pool(name="xq", bufs=3) as xq_pool,
            tc.tile_pool(name="xqt", bufs=8) as xqt_pool,
            tc.tile_pool(name="osb", bufs=3) as osb_pool,
            tc.tile_pool(name="sqb", bufs=3) as sqb_pool,
            tc.tile_pool(name="sc", bufs=12) as sc_pool,
            tc.tile_pool(name="mmps", bufs=3, space="PSUM") as mm_pool,
            tc.tile_pool(name="tps", bufs=2, space="PSUM") as tps_pool,
            tc.tile_pool(name="dram", bufs=2, space="DRAM") as dram_pool,
        ):
            ident = const_pool.tile([128, 128], dt.bfloat16)
            make_identity(nc, ident[:])

            wqT_cs = [wres_pool.tile([128, KT * 512], dt.bfloat16,
                                     name=f"wqT{c}") for c in range(OC)]
            wqT3_cs = [w[:].rearrange("p (j o) -> p j o", o=512) for w in wqT_cs]
            ws = wres_pool.tile([128, 1], dt.float32)
            halfws = wres_pool.tile([128, 1], dt.float32)
            neghws = wres_pool.tile([128, 1], dt.float32)

            def x_quant(t, head, order_after=None):
                # DMA + per-token scale + exact quantization + xbar transpose
                # for token tile t; returns (xqT, g). Only the tiny g op
                # depends on the collective result ws. order_after adds a
                # scheduling-only edge so steady-tile DVE work cannot crowd
                # out the post-AllReduce weight-quantization chain.
                xt = x_pool.tile([128, K], dt.float32, tag="x", name="x")
                nc.sync.dma_start(xt[:], x_ap[128 * t:128 * (t + 1), :])

                amax = sc_pool.tile([128, 1], dt.float32, tag="amax",
                                    name="amax")
                am_inst = nc.vector.tensor_reduce(amax[:], xt[:],
                                                  axis=mybir.AxisListType.X,
                                                  op=Alu.max,
                                                  apply_absolute_value=True)
                if order_after is not None:
                    add_dep_helper(am_inst.ins, order_after.ins, sync=False,
                                   reason="steady x work after wq chain")
                am2 = sc_pool.tile([128, 1], dt.float32, tag="am2", name="am2")
                nc.vector.tensor_scalar_max(am2[:], amax[:], 1e-5)
                rinv = sc_pool.tile([128, 1], dt.float32, tag="rinv",
                                    name="rinv")
                nc.vector.reciprocal(rinv[:], am2[:])
                rs = sc_pool.tile([128, 1], dt.float32, tag="rs", name="rs")
                nc.vector.tensor_scalar_mul(rs[:], rinv[:], float(max_val))
                g = sc_pool.tile([128, 1], dt.float32, tag="g", name="g")
                nc.vector.tensor_tensor(g[:], ws[:], rinv[:], op=Alu.mult)

                # x_q = rint(fl(x * rs)): fp32 product on ACT, then RNE to
                # integer via +C/-C on DVE, cast to exact bf16 integers
                xqf = xqf_pool.tile([128, K], dt.float32, tag="xqf", name="xqf")
                nc.scalar.activation(xqf[:], xt[:],
                                     mybir.ActivationFunctionType.Copy,
                                     scale=rs[:])
                xq = xq_pool.tile([128, K], dt.bfloat16, tag="xq", name="xq")
                nc.vector.tensor_scalar(xq[:], xqf[:], C_MAGIC, C_MAGIC,
                                        op0=Alu.add, op1=Alu.subtract)

                # PE transpose xq -> xqT [128, KT*128] bf16 (k on partitions);
                # the PE interleaves these with GEMM matmuls without breaking
                # the stream (a DMA-xbar transpose here stalls the PE's SBUF
                # reads and is serialized against the collective)
                xqT = xqt_pool.tile([128, KT * 128], dt.bfloat16, tag="xqT",
                                    name="xqT")
                half = KT // 2
                for hh in range(2):
                    ps = tps_pool.tile([128, half * 128], dt.bfloat16,
                                       tag="tps", name="tps")
                    for q in range(half):
                        j = hh * half + q
                        nc.tensor.transpose(
                            ps[:, 128 * q:128 * (q + 1)],
                            xq[:, 128 * j:128 * (j + 1)], ident[:])
                    dst = xqT[:, 128 * half * hh:128 * half * (hh + 1)]
                    if hh == 0:
                        nc.scalar.copy(dst, ps[:])
                    else:
                        nc.vector.tensor_copy(dst, ps[:])
                return xqT, g

            def gemm(c, xqT):
                ps = mm_pool.tile([128, 512], dt.float32, tag=f"mm{c}",
                                  name=f"mm{c}")
                for j in range(KT):
                    nc.tensor.matmul(ps[:], xqT[:, 128 * j:128 * (j + 1)],
                                     wqT3_cs[c][:, j, :],
                                     start=(j == 0), stop=(j == KT - 1))
                return ps

            def gemm2(xqTv):
                # chunk-major: all 16 k-steps into one psum bank, then the
                # next bank (alternating banks per-MM makes the PE micro-idle)
                pss = []
                for c in range(OC):
                    ps = mm_pool.tile([128, 512], dt.float32, tag=f"mm{c}",
                                      name=f"mm{c}")
                    for j in range(KT):
                        nc.tensor.matmul(ps[:], xqTv[:, 128 * j:128 * (j + 1)],
                                         wqT3_cs[c][:, j, :],
                                         start=(j == 0), stop=(j == KT - 1))
                    pss.append(ps)
                return pss

            def drain(t, c, ps, g):
                # out chunk = (relu(g*psum))^2 as [128, 512]. The relu runs
                # on ACT so the PSUM-bank release never queues behind the
                # DVE's amax/round work for future tiles; the square runs on
                # DVE from SBUF.
                osbh = osb_pool.tile([128, 512], dt.float32, tag="osbh",
                                     name="osbh")
                nc.scalar.activation(osbh[:], ps[:],
                                     mybir.ActivationFunctionType.Relu,
                                     scale=g[:])
                sqh = sqb_pool.tile([128, 512], dt.float32, tag="sqh",
                                    name="sqh")
                nc.vector.tensor_tensor(sqh[:], osbh[:], osbh[:], op=Alu.mult)
                nc.sync.dma_start(
                    out_ap[128 * t:128 * (t + 1), 512 * c:512 * (c + 1)],
                    sqh[:])

            # ------------- weight phase (staging pools freed after) -------------
            with (
                tc.tile_pool(name="w32", bufs=1) as w32_pool,
                tc.tile_pool(name="wq", bufs=2) as wq_pool,
            ):
                w32s = [w32_pool.tile([128, K], dt.float32, name=f"w32_{r}")
                        for r in range(OT)]
                wpart = wres_pool.tile([128, 2 * OT], dt.float32)

                with tc.high_priority():
                    # pass 1: stream w tiles (kept resident), |w| partial
                    # sums; half-tile DMAs so the first reduces start sooner
                    KH = K // 2
                    for r in range(OT):
                        for h in range(2):
                            nc.scalar.dma_start(
                                w32s[r][:, KH * h:KH * (h + 1)],
                                w_ap[128 * r:128 * (r + 1),
                                     KH * h:KH * (h + 1)])
                            nc.vector.tensor_reduce(
                                wpart[:, 2 * r + h:2 * r + h + 1],
                                w32s[r][:, KH * h:KH * (h + 1)],
                                axis=mybir.AxisListType.X,
                                op=Alu.add, apply_absolute_value=True)
                    wpart1 = wres_pool.tile([128, 1], dt.float32)
                    nc.vector.tensor_reduce(wpart1[:], wpart[:],
                                            axis=mybir.AxisListType.X,
                                            op=Alu.add)
                    wtot = wres_pool.tile([128, 1], dt.float32)
                    nc.gpsimd.partition_all_reduce(
                        wtot[:], wpart1[:], channels=128,
                        reduce_op=bass_isa.ReduceOp.add)

                    # scalar AllGather across the 8 cores via DRAM bounce bufs
                    # (cheaper floor than AllReduce for 4 bytes); each core
                    # sums the 8 gathered values locally. Bounce DMAs ride the
                    # fast HWDGE rings; gpsimd only triggers the collective.
                    cc_in = dram_pool.tile([1, 1], dt.float32)
                    cc_out = dram_pool.tile([NCORES, 1], dt.float32)
                    nc.sync.dma_start(cc_in[:], wtot[0:1, 0:1])
                    nc.gpsimd.collective_compute(
                        "AllGather", Alu.bypass,
                        replica_groups=[list(range(NCORES))],
                        ins=[cc_in.opt()], outs=[cc_out.opt()])
                    wsum8 = wres_pool.tile([128, NCORES], dt.float32)
                    nc.sync.dma_start(
                        wsum8[:],
                        cc_out[:].rearrange("a b -> b a").broadcast_to(
                            [128, NCORES]))
                    wsum_bc = wres_pool.tile([128, 1], dt.float32)
                    nc.vector.tensor_reduce(wsum_bc[:], wsum8[:],
                                            axis=mybir.AxisListType.X,
                                            op=Alu.add)

                    ws_inst = nc.vector.tensor_scalar_mul(ws[:], wsum_bc[:],
                                                          1.0 / n_w_elem)
                    nc.vector.tensor_scalar_mul(halfws[:], ws[:], 0.5)
                    nc.vector.tensor_scalar_mul(neghws[:], ws[:], -0.5)

                def w_quant(r):
                    # w_q = (w > 0.5ws) - (w < -0.5ws) via exact strict
                    # compares (0/1 results are exact in bf16), then xbar
                    # transpose into the wqT chunk
                    tp = wq_pool.tile([128, K], dt.bfloat16, tag="wtp", bufs=1)
                    nc.vector.tensor_scalar(tp[:], w32s[r][:], halfws[:], None,
                                            op0=Alu.is_gt)
                    tn = wq_pool.tile([128, K], dt.bfloat16, tag="wtn", bufs=1)
                    nc.vector.tensor_scalar(tn[:], w32s[r][:], neghws[:], None,
                                            op0=Alu.is_lt)
                    wq = wq_pool.tile([128, K], dt.bfloat16, tag="wq")
                    tt = nc.vector.tensor_tensor(wq[:], tp[:], tn[:],
                                                 op=Alu.subtract)
                    c, rr = r // 4, r % 4
                    xb = nc.scalar.dma_start_transpose(
                        wqT3_cs[c][:, :, 128 * rr:128 * (rr + 1)], wq[:])
                    return tt, xb

                # chunk 0 of the weights first, at high priority
                with tc.high_priority():
                    wq_insts = [w_quant(r) for r in range(4)]

                # head of x tiles, processed in the collective's shadow (only
                # each head tile's tiny g op actually waits for ws)
                head_tiles = [x_quant(t, head=True) for t in range(HEAD)]

                # remaining weight chunks
                with tc.high_priority():
                    wq_insts += [w_quant(r) for r in range(4, OT)]
                wq_last = wq_insts[-1][0]

                # ---------------- main loop over token tiles ----------------
                # emitted inside the weight-phase pool scope for the head part
                deferred = []
                for t in range(HEAD_C0):
                    xqT, g = head_tiles[t]
                    ps0 = gemm(0, xqT)
                    drain(t, 0, ps0, g)
                    deferred.append((t, xqT, g))

            # flush deferred chunk-1.. work, then steady state
            for (t, xqT, g) in deferred:
                for c in range(1, OC):
                    psc = gemm(c, xqT)
                    drain(t, c, psc, g)

            for t in range(HEAD_C0, TT):
                if t < HEAD:
                    xqT, g = head_tiles[t]
                else:
                    xqT, g = x_quant(t, head=False, order_after=wq_last)
                pss = gemm2(xqT[:])
                for c in range(OC):
                    drain(t, c, pss[c], g)

    nc.compile()
    return nc


def _get_nc(T, K, O, max_val):
    key = (T, K, O, max_val)
    if key not in _NC_CACHE:
        _NC_CACHE[key] = _build(T, K, O, max_val)
    return _NC_CACHE[key]


def kernel(x, weight, bits=8):
    global LAST_RESULTS
    x = np.asarray(x, dtype=np.float32)
    weight = np.asarray(weight, dtype=np.float32)
    bits = int(bits)
    max_val = (1 << (bits - 1)) - 1

    lead_shape = x.shape[:-1]
    K = x.shape[-1]
    T = int(np.prod(lead_shape))
    O_total, K_w = weight.shape
    assert K == K_w and O_total % NCORES == 0
    O = O_total // NCORES

    nc = _get_nc(T, K, O, max_val)

    x2 = np.ascontiguousarray(x.reshape(T, K))
    in_maps = [{"x": x2, "w": np.ascontiguousarray(weight[i * O:(i + 1) * O])}
               for i in range(NCORES)]
    res = run_bass_kernel_spmd(nc, in_maps, list(range(NCORES)))
    LAST_RESULTS = res

    out = np.concatenate([res.results[i]["out"] for i in range(NCORES)], axis=1)
    return out.reshape(*lead_shape, O_total)
